# revision 10
# baseline (speedup 1.0000x reference)
"""LCALayer Trainium2 kernel.

Reference math (per row i of [N, 8] tensors):
    active_i = all(|activities_i,:| < 1.0)                       -> [N, 1]
    recur_i  = sum_j (activities @ gamma)_ij = -0.7 * sum_k activities_ik
    pre_i    = pre_activities_i
               + active_i * (0.01*input_i - 0.001*pre_activities_i
                             + 0.01*recur_i + sqrt(0.001)*noise_i)
    act_i    = relu(pre_i)
Outputs: (pre, act, active).

Sharding: pure data parallel on the N axis across 8 NeuronCores; each core
processes 250,000 rows.  Per-core layout: rows are split partition-major
(partition p owns a contiguous run of rows) so every DMA moves long
contiguous spans; per-row reductions over D=8 run on the free axis via 3D
access patterns [P, R, 8] -> [P, R].
"""

import sys
from contextlib import ExitStack

import numpy as np

sys.path.insert(0, "/opt/trn_rl_repo")

import concourse.bass as bass  # noqa: E402
import concourse.tile as tile  # noqa: E402
from concourse import bacc, mybir  # noqa: E402
from concourse.bass_utils import run_bass_kernel_spmd  # noqa: E402

N_TOTAL = 2_000_000
N_DIM = 8
N_CORES = 8
N_PER_CORE = N_TOTAL // N_CORES  # 250_000

# LCALayer constants
DT_STEP = 0.01
LEAK = 0.1
COMPETITION = 0.1
SELF_EXCITATION = 0.0
SQRT_STEP = float(np.sqrt(0.001))
# gamma row-sum: self_excitation + (D-1) * (-competition)
GAMMA_ROWSUM = SELF_EXCITATION - COMPETITION * (N_DIM - 1)  # -0.7
A_INP = DT_STEP               # 0.01   * input
A_PRE = -LEAK * DT_STEP       # -0.001 * pre_activities
A_NOI = SQRT_STEP             # sqrt(0.001) * noise
# c (per-row addend) = 0.01 * recur = DT*GAMMA_ROWSUM * rowsum
C_ROW = DT_STEP * GAMMA_ROWSUM  # -0.007

F32 = mybir.dt.float32
ALU = mybir.AluOpType
AXIS_X = mybir.AxisListType.X


def _emit_block(nc, pools, views, part, rows, col0):
    """Emit ops for one tile: `part` partitions x `rows` rows/partition,
    starting at row-column `col0` within each partition's row run.

    Engine split (per full-tile pass costs): DVE gets the two row
    reductions + three fused stt passes; Pool gets two tensor_tensor
    passes; ACT pre-scales noise/input and does the final relu.  Loads
    issue on the SP HWDGE ring, stores on the ACT HWDGE ring so stores
    waiting on compute never head-of-line-block later tiles' loads."""
    pin, ptmp, pout, prow, pact = pools
    fl = rows * N_DIM
    c0e = col0 * N_DIM  # element offset within a partition's span

    t_act = pin.tile([part, fl], F32, tag="t_act")
    nc.sync.dma_start(t_act[:], views["activities"][:part, c0e:c0e + fl])
    t_noi = pin.tile([part, fl], F32, tag="t_noi")
    nc.sync.dma_start(t_noi[:], views["noise_raw"][:part, c0e:c0e + fl])
    t_inp = pin.tile([part, fl], F32, tag="t_inp")
    nc.sync.dma_start(t_inp[:], views["input"][:part, c0e:c0e + fl])
    t_pre = pin.tile([part, fl], F32, tag="t_pre")
    nc.sync.dma_start(t_pre[:], views["pre_activities"][:part, c0e:c0e + fl])

    act3 = t_act[:].rearrange("p (r d) -> p r d", d=N_DIM)

    # Row stats on DVE: max|a| and sum(a) over the inner D=8 axis
    r_max = prow.tile([part, rows], F32, tag="r_max")
    nc.vector.reduce_max(r_max[:], act3, axis=AXIS_X, apply_absolute_value=True)
    r_sum = prow.tile([part, rows], F32, tag="r_sum")
    nc.vector.reduce_sum(r_sum[:], act3, axis=AXIS_X)

    # active = (max|a| < 1.0) as 1.0/0.0 (output tensor)
    r_acv = prow.tile([part, rows], F32, tag="r_acv")
    nc.vector.tensor_scalar(r_acv[:], r_max[:], 1.0, None, op0=ALU.is_lt)

    rmax_b = r_max[:].unsqueeze(2).broadcast_to([part, rows, N_DIM])
    rsum_b = r_sum[:].unsqueeze(2).broadcast_to([part, rows, N_DIM])

    # ACT pre-scales (Copy is resident in every ACT table set)
    noi_s = pact.tile([part, fl], F32, tag="noi_s")
    nc.scalar.activation(noi_s[:], t_noi[:],
                         mybir.ActivationFunctionType.Copy, scale=A_NOI)
    inp_s = pact.tile([part, fl], F32, tag="inp_s")
    nc.scalar.activation(inp_s[:], t_inp[:],
                         mybir.ActivationFunctionType.Copy, scale=A_INP)

    # t1 = C_ROW*rowsum + A_NOI*noise ; t2 = A_PRE*pre + A_INP*inp
    t1 = ptmp.tile([part, fl], F32, tag="t1")
    t1_3 = t1[:].rearrange("p (r d) -> p r d", d=N_DIM)
    noi3 = noi_s[:].rearrange("p (r d) -> p r d", d=N_DIM)
    nc.vector.scalar_tensor_tensor(
        t1_3, rsum_b, C_ROW, noi3, op0=ALU.mult, op1=ALU.add)
    t2 = ptmp.tile([part, fl], F32, tag="t2")
    nc.vector.scalar_tensor_tensor(
        t2[:], t_pre[:], A_PRE, inp_s[:], op0=ALU.mult, op1=ALU.add)

    # t1 = (t1 + t2) * active   (active recomputed inline from rmax)
    nc.gpsimd.tensor_tensor(t1[:], t1[:], t2[:], op=ALU.add)
    nc.vector.scalar_tensor_tensor(
        t1_3, rmax_b, 1.0, t1_3, op0=ALU.is_lt, op1=ALU.mult)

    # pre_out = pre + t1 (Pool) ; act_out = relu(pre_out) (ACT)
    t_preo = pout.tile([part, fl], F32, tag="t_preo")
    nc.gpsimd.tensor_tensor(t_preo[:], t_pre[:], t1[:], op=ALU.add)
    t_acto = pout.tile([part, fl], F32, tag="t_acto")
    nc.scalar.activation(t_acto[:], t_preo[:], mybir.ActivationFunctionType.Relu)

    nc.scalar.dma_start(views["pre_out"][:part, c0e:c0e + fl], t_preo[:])
    nc.scalar.dma_start(views["act_out"][:part, c0e:c0e + fl], t_acto[:])
    nc.scalar.dma_start(views["active_out"][:part, col0:col0 + rows], r_acv[:])


def build_program(n_rows=N_PER_CORE, tile_rows=217):
    """Build the per-core Bass program. All cores run the identical program
    on their own 250k-row shard (pure data parallel, no collectives)."""
    nc = bacc.Bacc(
        "TRN2", target_bir_lowering=False, debug=False, num_devices=N_CORES)

    tensors = {}
    for name in ("input", "pre_activities", "activities", "noise_raw"):
        tensors[name] = nc.declare_dram_parameter(
            name, [n_rows, N_DIM], F32, isOutput=False)
    tensors["pre_out"] = nc.declare_dram_parameter(
        "pre_out", [n_rows, N_DIM], F32, isOutput=True)
    tensors["act_out"] = nc.declare_dram_parameter(
        "act_out", [n_rows, N_DIM], F32, isOutput=True)
    tensors["active_out"] = nc.declare_dram_parameter(
        "active_out", [n_rows, 1], F32, isOutput=True)

    rows_pp = n_rows // 128          # rows per partition (main part)
    main_rows = rows_pp * 128
    rem_rows = n_rows - main_rows    # handled as a small [rem, 8] block
    n_tiles = rows_pp // tile_rows
    tail_rows = rows_pp - n_tiles * tile_rows
    assert rem_rows <= 128

    # Tile schedule: big tiles, then taper the final big tile into small
    # ones so the pipeline drains quickly after the last load lands
    # (short final compute chain + small final stores).
    tiles = [tile_rows] * n_tiles
    if tail_rows:
        tiles.append(tail_rows)
    if tiles and tiles[-1] > 62:
        last = tiles.pop()
        q, r = divmod(last, 31)
        taper = [31] * q + ([r] if r else [])
        tiles.extend(taper)

    def part_major(handle, width, nrows, parts):
        # [nrows*width] flat -> [parts, (nrows//parts)*width], partition-major
        return (handle[:].flatten()[0:nrows * width]
                .rearrange("(p f) -> p f", p=parts))

    views = {}
    rem_views = {}
    for name, h in tensors.items():
        width = 1 if name == "active_out" else N_DIM
        views[name] = part_major(h, width, main_rows, 128)
        if rem_rows:
            rem_views[name] = (
                h[:].flatten()[main_rows * width: n_rows * width]
                .rearrange("(p f) -> p f", p=rem_rows))

    with tile.TileContext(nc) as tc, ExitStack() as ctx:
        pin = ctx.enter_context(tc.tile_pool(name="pin", bufs=3))
        ptmp = ctx.enter_context(tc.tile_pool(name="ptmp", bufs=2))
        pout = ctx.enter_context(tc.tile_pool(name="pout", bufs=2))
        prow = ctx.enter_context(tc.tile_pool(name="prow", bufs=3))
        pact = ctx.enter_context(tc.tile_pool(name="pact", bufs=2))
        pools = (pin, ptmp, pout, prow, pact)

        col = 0
        for trows in tiles:
            _emit_block(nc, pools, views, 128, trows, col)
            col += trows
        assert col == rows_pp
        if rem_rows:
            _emit_block(nc, pools, rem_views, rem_rows, 1, 0)

    nc.compile()
    return nc


_PROGRAM_CACHE = {}


def _get_program():
    key = (N_PER_CORE,)
    if key not in _PROGRAM_CACHE:
        _PROGRAM_CACHE[key] = build_program()
    return _PROGRAM_CACHE[key]


def kernel(input, pre_activities, activities, noise_raw):
    nc = _get_program()
    shards = []
    for k in range(N_CORES):
        sl = slice(k * N_PER_CORE, (k + 1) * N_PER_CORE)
        shards.append({
            "input": np.ascontiguousarray(input[sl], dtype=np.float32),
            "pre_activities": np.ascontiguousarray(
                pre_activities[sl], dtype=np.float32),
            "activities": np.ascontiguousarray(activities[sl], dtype=np.float32),
            "noise_raw": np.ascontiguousarray(noise_raw[sl], dtype=np.float32),
        })
    res = run_bass_kernel_spmd(nc, shards, list(range(N_CORES)))
    pre = np.concatenate([r["pre_out"] for r in res.results], axis=0)
    act = np.concatenate([r["act_out"] for r in res.results], axis=0)
    active = np.concatenate([r["active_out"] for r in res.results], axis=0)
    return pre, act, active


# revision 11
# speedup vs baseline: 1.0701x; 1.0701x over previous
"""LCALayer Trainium2 kernel.

Reference math (per row i of [N, 8] tensors):
    active_i = all(|activities_i,:| < 1.0)                       -> [N, 1]
    recur_i  = sum_j (activities @ gamma)_ij = -0.7 * sum_k activities_ik
    pre_i    = pre_activities_i
               + active_i * (0.01*input_i - 0.001*pre_activities_i
                             + 0.01*recur_i + sqrt(0.001)*noise_i)
    act_i    = relu(pre_i)
Outputs: (pre, act, active).

Sharding: pure data parallel on the N axis across 8 NeuronCores; each core
processes 250,000 rows.  Per-core layout: rows are split partition-major
(partition p owns a contiguous run of rows) so every DMA moves long
contiguous spans; per-row reductions over D=8 run on the free axis via 3D
access patterns [P, R, 8] -> [P, R].
"""

import sys
from contextlib import ExitStack

import numpy as np

sys.path.insert(0, "/opt/trn_rl_repo")

import concourse.bass as bass  # noqa: E402
import concourse.tile as tile  # noqa: E402
from concourse import bacc, mybir  # noqa: E402
from concourse.bass_utils import run_bass_kernel_spmd  # noqa: E402

N_TOTAL = 2_000_000
N_DIM = 8
N_CORES = 8
N_PER_CORE = N_TOTAL // N_CORES  # 250_000

# LCALayer constants
DT_STEP = 0.01
LEAK = 0.1
COMPETITION = 0.1
SELF_EXCITATION = 0.0
SQRT_STEP = float(np.sqrt(0.001))
# gamma row-sum: self_excitation + (D-1) * (-competition)
GAMMA_ROWSUM = SELF_EXCITATION - COMPETITION * (N_DIM - 1)  # -0.7
A_INP = DT_STEP               # 0.01   * input
A_PRE = -LEAK * DT_STEP       # -0.001 * pre_activities
A_NOI = SQRT_STEP             # sqrt(0.001) * noise
# c (per-row addend) = 0.01 * recur = DT*GAMMA_ROWSUM * rowsum
C_ROW = DT_STEP * GAMMA_ROWSUM  # -0.007

F32 = mybir.dt.float32
ALU = mybir.AluOpType
AXIS_X = mybir.AxisListType.X


def _emit_block(nc, pools, views, part, rows, col0):
    """Emit ops for one tile: `part` partitions x `rows` rows/partition,
    starting at row-column `col0` within each partition's row run.

    Engine split (per full-tile pass costs): DVE gets the two row
    reductions + three fused stt passes; Pool gets two tensor_tensor
    passes; ACT pre-scales noise/input and does the final relu.  Loads
    issue on the SP HWDGE ring, stores on the ACT HWDGE ring so stores
    waiting on compute never head-of-line-block later tiles' loads."""
    pin, ptmp, pout, prow, pact = pools
    fl = rows * N_DIM
    c0e = col0 * N_DIM  # element offset within a partition's span

    t_act = pin.tile([part, fl], F32, tag="t_act")
    nc.sync.dma_start(t_act[:], views["activities"][:part, c0e:c0e + fl])
    t_noi = pin.tile([part, fl], F32, tag="t_noi")
    nc.sync.dma_start(t_noi[:], views["noise_raw"][:part, c0e:c0e + fl])
    t_inp = pin.tile([part, fl], F32, tag="t_inp")
    nc.sync.dma_start(t_inp[:], views["input"][:part, c0e:c0e + fl])
    t_pre = pin.tile([part, fl], F32, tag="t_pre")
    nc.sync.dma_start(t_pre[:], views["pre_activities"][:part, c0e:c0e + fl])

    act3 = t_act[:].rearrange("p (r d) -> p r d", d=N_DIM)

    # Row stats on DVE: max|a| and sum(a) over the inner D=8 axis
    r_max = prow.tile([part, rows], F32, tag="r_max")
    nc.vector.reduce_max(r_max[:], act3, axis=AXIS_X, apply_absolute_value=True)
    r_sum = prow.tile([part, rows], F32, tag="r_sum")
    nc.vector.reduce_sum(r_sum[:], act3, axis=AXIS_X)

    # active = (max|a| < 1.0) as 1.0/0.0 (output tensor)
    r_acv = prow.tile([part, rows], F32, tag="r_acv")
    nc.vector.tensor_scalar(r_acv[:], r_max[:], 1.0, None, op0=ALU.is_lt)

    rmax_b = r_max[:].unsqueeze(2).broadcast_to([part, rows, N_DIM])
    rsum_b = r_sum[:].unsqueeze(2).broadcast_to([part, rows, N_DIM])

    # ACT pre-scales (Copy is resident in every ACT table set)
    noi_s = pact.tile([part, fl], F32, tag="noi_s")
    nc.scalar.activation(noi_s[:], t_noi[:],
                         mybir.ActivationFunctionType.Copy, scale=A_NOI)
    inp_s = pact.tile([part, fl], F32, tag="inp_s")
    nc.scalar.activation(inp_s[:], t_inp[:],
                         mybir.ActivationFunctionType.Copy, scale=A_INP)

    # t1 = C_ROW*rowsum + A_NOI*noise ; t2 = A_PRE*pre + A_INP*inp
    t1 = ptmp.tile([part, fl], F32, tag="t1")
    t1_3 = t1[:].rearrange("p (r d) -> p r d", d=N_DIM)
    noi3 = noi_s[:].rearrange("p (r d) -> p r d", d=N_DIM)
    nc.vector.scalar_tensor_tensor(
        t1_3, rsum_b, C_ROW, noi3, op0=ALU.mult, op1=ALU.add)
    t2 = ptmp.tile([part, fl], F32, tag="t2")
    nc.vector.scalar_tensor_tensor(
        t2[:], t_pre[:], A_PRE, inp_s[:], op0=ALU.mult, op1=ALU.add)

    # t1 = (t1 + t2) * active   (active recomputed inline from rmax)
    nc.gpsimd.tensor_tensor(t1[:], t1[:], t2[:], op=ALU.add)
    nc.vector.scalar_tensor_tensor(
        t1_3, rmax_b, 1.0, t1_3, op0=ALU.is_lt, op1=ALU.mult)

    # pre_out = pre + t1 (Pool) ; act_out = relu(pre_out) (ACT)
    t_preo = pout.tile([part, fl], F32, tag="t_preo")
    nc.gpsimd.tensor_tensor(t_preo[:], t_pre[:], t1[:], op=ALU.add)
    t_acto = pout.tile([part, fl], F32, tag="t_acto")
    nc.scalar.activation(t_acto[:], t_preo[:], mybir.ActivationFunctionType.Relu)

    nc.scalar.dma_start(views["pre_out"][:part, c0e:c0e + fl], t_preo[:])
    nc.scalar.dma_start(views["act_out"][:part, c0e:c0e + fl], t_acto[:])
    nc.scalar.dma_start(views["active_out"][:part, col0:col0 + rows], r_acv[:])


def build_program(n_rows=N_PER_CORE, tile_rows=217):
    """Build the per-core Bass program. All cores run the identical program
    on their own 250k-row shard (pure data parallel, no collectives)."""
    nc = bacc.Bacc(
        "TRN2", target_bir_lowering=False, debug=False, num_devices=N_CORES)

    tensors = {}
    for name in ("input", "pre_activities", "activities", "noise_raw"):
        tensors[name] = nc.declare_dram_parameter(
            name, [n_rows, N_DIM], F32, isOutput=False)
    tensors["pre_out"] = nc.declare_dram_parameter(
        "pre_out", [n_rows, N_DIM], F32, isOutput=True)
    tensors["act_out"] = nc.declare_dram_parameter(
        "act_out", [n_rows, N_DIM], F32, isOutput=True)
    tensors["active_out"] = nc.declare_dram_parameter(
        "active_out", [n_rows, 1], F32, isOutput=True)

    rows_pp = n_rows // 128          # rows per partition (main part)
    main_rows = rows_pp * 128
    rem_rows = n_rows - main_rows    # handled as a small [rem, 8] block
    n_tiles = rows_pp // tile_rows
    tail_rows = rows_pp - n_tiles * tile_rows
    assert rem_rows <= 128

    # Tile schedule: big tiles, with the final big tile split in half so
    # the pipeline drains quickly after the last load lands (shorter final
    # compute chain + smaller final stores).  Finer tapers fragment the
    # DMAs below ~4KB/partition and cost more than they save.
    tiles = [tile_rows] * n_tiles
    if tail_rows:
        tiles.append(tail_rows)
    if tiles and tiles[-1] > 62:
        last = tiles.pop()
        tiles.extend([(last + 1) // 2, last // 2])

    def part_major(handle, width, nrows, parts):
        # [nrows*width] flat -> [parts, (nrows//parts)*width], partition-major
        return (handle[:].flatten()[0:nrows * width]
                .rearrange("(p f) -> p f", p=parts))

    views = {}
    rem_views = {}
    for name, h in tensors.items():
        width = 1 if name == "active_out" else N_DIM
        views[name] = part_major(h, width, main_rows, 128)
        if rem_rows:
            rem_views[name] = (
                h[:].flatten()[main_rows * width: n_rows * width]
                .rearrange("(p f) -> p f", p=rem_rows))

    with tile.TileContext(nc) as tc, ExitStack() as ctx:
        pin = ctx.enter_context(tc.tile_pool(name="pin", bufs=3))
        ptmp = ctx.enter_context(tc.tile_pool(name="ptmp", bufs=2))
        pout = ctx.enter_context(tc.tile_pool(name="pout", bufs=2))
        prow = ctx.enter_context(tc.tile_pool(name="prow", bufs=3))
        pact = ctx.enter_context(tc.tile_pool(name="pact", bufs=2))
        pools = (pin, ptmp, pout, prow, pact)

        col = 0
        for trows in tiles:
            _emit_block(nc, pools, views, 128, trows, col)
            col += trows
        assert col == rows_pp
        if rem_rows:
            _emit_block(nc, pools, rem_views, rem_rows, 1, 0)

    nc.compile()
    return nc


_PROGRAM_CACHE = {}


def _get_program():
    key = (N_PER_CORE,)
    if key not in _PROGRAM_CACHE:
        _PROGRAM_CACHE[key] = build_program()
    return _PROGRAM_CACHE[key]


def kernel(input, pre_activities, activities, noise_raw):
    nc = _get_program()
    shards = []
    for k in range(N_CORES):
        sl = slice(k * N_PER_CORE, (k + 1) * N_PER_CORE)
        shards.append({
            "input": np.ascontiguousarray(input[sl], dtype=np.float32),
            "pre_activities": np.ascontiguousarray(
                pre_activities[sl], dtype=np.float32),
            "activities": np.ascontiguousarray(activities[sl], dtype=np.float32),
            "noise_raw": np.ascontiguousarray(noise_raw[sl], dtype=np.float32),
        })
    res = run_bass_kernel_spmd(nc, shards, list(range(N_CORES)))
    pre = np.concatenate([r["pre_out"] for r in res.results], axis=0)
    act = np.concatenate([r["act_out"] for r in res.results], axis=0)
    active = np.concatenate([r["active_out"] for r in res.results], axis=0)
    return pre, act, active


# revision 12
# speedup vs baseline: 1.1435x; 1.0686x over previous
"""LCALayer Trainium2 kernel.

Reference math (per row i of [N, 8] tensors):
    active_i = all(|activities_i,:| < 1.0)                       -> [N, 1]
    recur_i  = sum_j (activities @ gamma)_ij = -0.7 * sum_k activities_ik
    pre_i    = pre_activities_i
               + active_i * (0.01*input_i - 0.001*pre_activities_i
                             + 0.01*recur_i + sqrt(0.001)*noise_i)
    act_i    = relu(pre_i)
Outputs: (pre, act, active).

Sharding: pure data parallel on the N axis across 8 NeuronCores; each core
processes 250,000 rows.  Per-core layout: rows are split partition-major
(partition p owns a contiguous run of rows) so every DMA moves long
contiguous spans; per-row reductions over D=8 run on the free axis via 3D
access patterns [P, R, 8] -> [P, R].
"""

import sys
from contextlib import ExitStack

import numpy as np

sys.path.insert(0, "/opt/trn_rl_repo")

import concourse.bass as bass  # noqa: E402
import concourse.tile as tile  # noqa: E402
from concourse import bacc, mybir  # noqa: E402
from concourse.bass_utils import run_bass_kernel_spmd  # noqa: E402

N_TOTAL = 2_000_000
N_DIM = 8
N_CORES = 8
N_PER_CORE = N_TOTAL // N_CORES  # 250_000

# LCALayer constants
DT_STEP = 0.01
LEAK = 0.1
COMPETITION = 0.1
SELF_EXCITATION = 0.0
SQRT_STEP = float(np.sqrt(0.001))
# gamma row-sum: self_excitation + (D-1) * (-competition)
GAMMA_ROWSUM = SELF_EXCITATION - COMPETITION * (N_DIM - 1)  # -0.7
A_INP = DT_STEP               # 0.01   * input
A_PRE = -LEAK * DT_STEP       # -0.001 * pre_activities
A_NOI = SQRT_STEP             # sqrt(0.001) * noise
# c (per-row addend) = 0.01 * recur = DT*GAMMA_ROWSUM * rowsum
C_ROW = DT_STEP * GAMMA_ROWSUM  # -0.007

F32 = mybir.dt.float32
ALU = mybir.AluOpType
AXIS_X = mybir.AxisListType.X


def _emit_block(nc, pools, views, part, rows, col0):
    """Emit ops for one tile: `part` partitions x `rows` rows/partition,
    starting at row-column `col0` within each partition's row run.

    Engine split (per full-tile pass costs): DVE gets the two row
    reductions + three fused stt passes; Pool gets two tensor_tensor
    passes; ACT pre-scales noise/input and does the final relu.  Loads
    issue on the SP HWDGE ring, stores on the ACT HWDGE ring so stores
    waiting on compute never head-of-line-block later tiles' loads."""
    pin, ptmp, pout, prow, pact = pools
    fl = rows * N_DIM
    c0e = col0 * N_DIM  # element offset within a partition's span

    t_act = pin.tile([part, fl], F32, tag="t_act")
    nc.sync.dma_start(t_act[:], views["activities"][:part, c0e:c0e + fl])
    t_noi = pin.tile([part, fl], F32, tag="t_noi")
    nc.sync.dma_start(t_noi[:], views["noise_raw"][:part, c0e:c0e + fl])
    t_inp = pin.tile([part, fl], F32, tag="t_inp")
    nc.sync.dma_start(t_inp[:], views["input"][:part, c0e:c0e + fl])
    t_pre = pin.tile([part, fl], F32, tag="t_pre")
    nc.sync.dma_start(t_pre[:], views["pre_activities"][:part, c0e:c0e + fl])

    act3 = t_act[:].rearrange("p (r d) -> p r d", d=N_DIM)

    # Row stats on DVE: max|a| and sum(a) over the inner D=8 axis
    r_max = prow.tile([part, rows], F32, tag="r_max")
    nc.vector.reduce_max(r_max[:], act3, axis=AXIS_X, apply_absolute_value=True)
    r_sum = prow.tile([part, rows], F32, tag="r_sum")
    nc.vector.reduce_sum(r_sum[:], act3, axis=AXIS_X)

    # active = (max|a| < 1.0) as 1.0/0.0 (output tensor)
    r_acv = prow.tile([part, rows], F32, tag="r_acv")
    nc.vector.tensor_scalar(r_acv[:], r_max[:], 1.0, None, op0=ALU.is_lt)

    rmax_b = r_max[:].unsqueeze(2).broadcast_to([part, rows, N_DIM])
    rsum_b = r_sum[:].unsqueeze(2).broadcast_to([part, rows, N_DIM])

    # ACT pre-scales (Copy is resident in every ACT table set)
    noi_s = pact.tile([part, fl], F32, tag="noi_s")
    nc.scalar.activation(noi_s[:], t_noi[:],
                         mybir.ActivationFunctionType.Copy, scale=A_NOI)
    inp_s = pact.tile([part, fl], F32, tag="inp_s")
    nc.scalar.activation(inp_s[:], t_inp[:],
                         mybir.ActivationFunctionType.Copy, scale=A_INP)

    # t1 = C_ROW*rowsum + A_NOI*noise ; t2 = A_PRE*pre + A_INP*inp
    t1 = ptmp.tile([part, fl], F32, tag="t1")
    t1_3 = t1[:].rearrange("p (r d) -> p r d", d=N_DIM)
    noi3 = noi_s[:].rearrange("p (r d) -> p r d", d=N_DIM)
    nc.vector.scalar_tensor_tensor(
        t1_3, rsum_b, C_ROW, noi3, op0=ALU.mult, op1=ALU.add)
    t2 = ptmp.tile([part, fl], F32, tag="t2")
    nc.vector.scalar_tensor_tensor(
        t2[:], t_pre[:], A_PRE, inp_s[:], op0=ALU.mult, op1=ALU.add)

    # t1 = (t1 + t2) * active   (active recomputed inline from rmax)
    nc.gpsimd.tensor_tensor(t1[:], t1[:], t2[:], op=ALU.add)
    nc.vector.scalar_tensor_tensor(
        t1_3, rmax_b, 1.0, t1_3, op0=ALU.is_lt, op1=ALU.mult)

    # pre_out = pre + t1 (Pool) ; act_out = relu(pre_out) (ACT)
    t_preo = pout.tile([part, fl], F32, tag="t_preo")
    nc.gpsimd.tensor_tensor(t_preo[:], t_pre[:], t1[:], op=ALU.add)
    t_acto = pout.tile([part, fl], F32, tag="t_acto")
    nc.scalar.activation(t_acto[:], t_preo[:], mybir.ActivationFunctionType.Relu)

    nc.scalar.dma_start(views["pre_out"][:part, c0e:c0e + fl], t_preo[:])
    nc.scalar.dma_start(views["act_out"][:part, c0e:c0e + fl], t_acto[:])
    nc.scalar.dma_start(views["active_out"][:part, col0:col0 + rows], r_acv[:])


def build_program(n_rows=N_PER_CORE, tile_rows=217):
    """Build the per-core Bass program. All cores run the identical program
    on their own 250k-row shard (pure data parallel, no collectives)."""
    nc = bacc.Bacc(
        "TRN2", target_bir_lowering=False, debug=False, num_devices=N_CORES)

    tensors = {}
    for name in ("input", "pre_activities", "activities", "noise_raw"):
        tensors[name] = nc.declare_dram_parameter(
            name, [n_rows, N_DIM], F32, isOutput=False)
    tensors["pre_out"] = nc.declare_dram_parameter(
        "pre_out", [n_rows, N_DIM], F32, isOutput=True)
    tensors["act_out"] = nc.declare_dram_parameter(
        "act_out", [n_rows, N_DIM], F32, isOutput=True)
    tensors["active_out"] = nc.declare_dram_parameter(
        "active_out", [n_rows, 1], F32, isOutput=True)

    rows_pp = n_rows // 128          # rows per partition (main part)
    main_rows = rows_pp * 128
    rem_rows = n_rows - main_rows    # handled as a small [rem, 8] block
    n_tiles = rows_pp // tile_rows
    tail_rows = rows_pp - n_tiles * tile_rows
    assert rem_rows <= 128

    # Uniform tile schedule. Tapering the trailing tiles (even a single
    # 2-way split of the last tile) measured slower on HW: smaller DMAs
    # and the disrupted pipeline cost more than the shorter drain saves.
    tiles = [tile_rows] * n_tiles
    if tail_rows:
        tiles.append(tail_rows)

    def part_major(handle, width, nrows, parts):
        # [nrows*width] flat -> [parts, (nrows//parts)*width], partition-major
        return (handle[:].flatten()[0:nrows * width]
                .rearrange("(p f) -> p f", p=parts))

    views = {}
    rem_views = {}
    for name, h in tensors.items():
        width = 1 if name == "active_out" else N_DIM
        views[name] = part_major(h, width, main_rows, 128)
        if rem_rows:
            rem_views[name] = (
                h[:].flatten()[main_rows * width: n_rows * width]
                .rearrange("(p f) -> p f", p=rem_rows))

    with tile.TileContext(nc) as tc, ExitStack() as ctx:
        pin = ctx.enter_context(tc.tile_pool(name="pin", bufs=3))
        ptmp = ctx.enter_context(tc.tile_pool(name="ptmp", bufs=2))
        pout = ctx.enter_context(tc.tile_pool(name="pout", bufs=2))
        prow = ctx.enter_context(tc.tile_pool(name="prow", bufs=3))
        pact = ctx.enter_context(tc.tile_pool(name="pact", bufs=2))
        pools = (pin, ptmp, pout, prow, pact)

        col = 0
        for trows in tiles:
            _emit_block(nc, pools, views, 128, trows, col)
            col += trows
        assert col == rows_pp
        if rem_rows:
            _emit_block(nc, pools, rem_views, rem_rows, 1, 0)

    nc.compile()
    return nc


_PROGRAM_CACHE = {}


def _get_program():
    key = (N_PER_CORE,)
    if key not in _PROGRAM_CACHE:
        _PROGRAM_CACHE[key] = build_program()
    return _PROGRAM_CACHE[key]


def kernel(input, pre_activities, activities, noise_raw):
    nc = _get_program()
    shards = []
    for k in range(N_CORES):
        sl = slice(k * N_PER_CORE, (k + 1) * N_PER_CORE)
        shards.append({
            "input": np.ascontiguousarray(input[sl], dtype=np.float32),
            "pre_activities": np.ascontiguousarray(
                pre_activities[sl], dtype=np.float32),
            "activities": np.ascontiguousarray(activities[sl], dtype=np.float32),
            "noise_raw": np.ascontiguousarray(noise_raw[sl], dtype=np.float32),
        })
    res = run_bass_kernel_spmd(nc, shards, list(range(N_CORES)))
    pre = np.concatenate([r["pre_out"] for r in res.results], axis=0)
    act = np.concatenate([r["act_out"] for r in res.results], axis=0)
    active = np.concatenate([r["active_out"] for r in res.results], axis=0)
    return pre, act, active
